# revision 16
# baseline (speedup 1.0000x reference)
"""Self-contained TRN2 Bass kernel for nn_MultiHeadAttentionLayer
(GNN multi-head attention message passing), 8 NeuronCores.

kernel(**inputs) takes the FULL unsharded inputs (h, Wq, bq, Wk, bk, Wv,
bv, src, dst) as numpy arrays and returns the FULL [N, H, D] float32
output. Sharding: edges are partitioned by dst range across the 8 cores
(no collectives needed). Each core projects K/V for all nodes into 4
src-group tables (so per-edge K|V row gathers can start after 1/4 of
the projection), gathers rows with dma_gather round-robined over SWDGE
queues 1-3 (their descriptor generation runs asynchronously on separate
Q7 core pairs), computes scores/softmax weights on DVE/ACT, and
segment-sums into per-superblock SBUF accumulators via one-hot (fp8)
matmuls on the TensorEngine.
"""

from dataclasses import dataclass, field

import numpy as np
import ml_dtypes

import concourse.bass as bass
import concourse.tile as tile
from concourse import bacc, mybir
from concourse.bass import ts
from concourse.bass_utils import run_bass_kernel_spmd

BF16 = ml_dtypes.bfloat16
F32 = np.float32
AF = mybir.ActivationFunctionType
ALU = mybir.AluOpType

CH_CAP = 18          # max blocks per gather/compute chunk
NGRP = 4             # src groups (table quarters)
N_SWDGE_Q = 4        # SWDGE queues allocated (queue 0 unused for gathers:
                     # it is synchronous on the Pool engine; 1-3 are async)
QW = 6               # blocks per Q-gather PSUM piece


@dataclass
class Cfg:
    N: int
    IN: int
    H: int
    D: int
    n_cores: int = 8
    NPC: int = 0
    NT: int = 0
    NSB: int = 0
    CHG: list = field(default_factory=list)   # [j][g] blocks
    ASSIGN: list = field(default_factory=list)  # [core][pos] -> global sb

    @property
    def C(self):
        return self.H * self.D

    @property
    def KA(self):
        return self.IN // 128

    @property
    def NTG(self):
        return self.NT // NGRP

    @property
    def SBLK(self):
        return [sum(row) for row in self.CHG]


def make_cfg(N, IN, H, D, src, dst, n_cores=8):
    cfg = Cfg(N=N, IN=IN, H=H, D=D, n_cores=n_cores)
    cfg.NPC = -(-N // (n_cores * 128)) * 128
    cfg.NT = cfg.NPC * n_cores
    cfg.NSB = cfg.NPC // 128
    src = np.asarray(src)
    dst = np.asarray(dst)
    gsb = dst // 128                       # global super-block of each edge
    nsb_tot = cfg.NSB * n_cores
    grp = np.minimum(src // cfg.NTG, NGRP - 1)
    counts = np.zeros((nsb_tot, NGRP), dtype=np.int64)
    np.add.at(counts, (np.minimum(gsb, nsb_tot - 1), grp), 1)
    # balanced assignment: position k across cores gets super-blocks of
    # similar total size -> minimal max-over-cores padding
    order = np.argsort(-counts.sum(axis=1), kind="stable")
    cfg.ASSIGN = [[int(order[k * n_cores + i]) for k in range(cfg.NSB)]
                  for i in range(n_cores)]
    cfg.CHG = []
    for k in range(cfg.NSB):
        g_sbs = order[k * n_cores:(k + 1) * n_cores]
        cfg.CHG.append([max(1, int(-(-counts[g_sbs, g].max() // 128)))
                        for g in range(NGRP)])
    return cfg


def chunk_plan(cfg):
    """[(j, g, cb, CH, key)] in canonical (j, g) order."""
    plan = []
    key = 0
    for j in range(cfg.NSB):
        for g in range(NGRP):
            CHG = cfg.CHG[j][g]
            b0 = 0
            while b0 < CHG:
                plan.append((j, g, b0, min(CH_CAP, CHG - b0), key))
                key += 1
                b0 += CH_CAP
    return plan


def _wrap16(idx, epb):
    base = idx.reshape(epb // 16, 16).T.astype(np.int16)
    return np.tile(base, (8, 1))


def prep(cfg: Cfg, h, Wq, bq, Wk, bk, Wv, bv, src, dst):
    N, IN, H, D, C = cfg.N, cfg.IN, cfg.H, cfg.D, cfg.C
    scale = 1.0 / np.sqrt(np.float32(D))

    hT = np.zeros((IN, cfg.NT), dtype=BF16)
    hT[:, :N] = np.asarray(h).T.astype(BF16)
    wkv = np.concatenate([np.asarray(Wk), np.asarray(Wv)], axis=1).astype(BF16)
    bkv = np.concatenate([np.asarray(bk), np.asarray(bv)])[None, :].astype(BF16)
    wq = (np.asarray(Wq) * scale).astype(BF16)
    bqs = (np.asarray(bq) * scale)[None, :].astype(BF16)

    src = np.asarray(src).astype(np.int64)
    dst = np.asarray(dst).astype(np.int64)

    sum_blk = sum(cfg.SBLK)
    sum_epb = sum_blk * 128

    gsb_of = dst // 128
    grp_of = np.minimum(src // cfg.NTG, NGRP - 1)
    in_maps = []
    for i in range(cfg.n_cores):
        srcidx = np.zeros(sum_epb, dtype=np.int64)
        ld = np.full((sum_blk, 128), 255, dtype=np.int64)
        off_e = 0
        off_b = 0
        for j in range(cfg.NSB):
            g_sb = cfg.ASSIGN[i][j]
            insb = gsb_of == g_sb
            es, ed, eg = src[insb], dst[insb] - g_sb * 128, grp_of[insb]
            for g in range(NGRP):
                chg = cfg.CHG[j][g]
                gsel = eg == g
                gidx = es[gsel] - g * cfg.NTG
                cnt = gidx.shape[0]
                epb = chg * 128
                assert cnt <= epb, (i, j, g, cnt, epb)
                srcidx[off_e:off_e + cnt] = gidx
                ldj = np.full(epb, 255, dtype=np.int64)
                ldj[:cnt] = ed[gsel]
                ld[off_b:off_b + chg, :] = ldj.reshape(chg, 128)
                off_e += epb
                off_b += chg

        srcw_parts = []
        off = 0
        for j in range(cfg.NSB):
            for g in range(NGRP):
                epb = cfg.CHG[j][g] * 128
                srcw_parts.append(_wrap16(srcidx[off:off + epb], epb))
                off += epb
        srcw = np.concatenate(srcw_parts, axis=1)

        # one-hot dst matrices in fp8 (0/1 exact): halves their DMA vs bf16
        marange = np.arange(128, dtype=np.int64)
        onehot = (ld[:, :, None] == marange[None, None, :])       # [bb, e, m]
        FP8 = mybir.dt.np(mybir.dt.float8e4)
        Sh = np.ascontiguousarray(onehot.transpose(1, 0, 2)).astype(FP8)
        ShT = np.ascontiguousarray(onehot.transpose(2, 0, 1)).astype(FP8)

        cols = np.concatenate(
            [np.arange(cfg.ASSIGN[i][j] * 128, cfg.ASSIGN[i][j] * 128 + 128)
             for j in range(cfg.NSB)])
        in_maps.append({
            "hT": hT,
            "hTq": np.ascontiguousarray(hT[:, cols]),
            "wkv": wkv, "bkv": bkv, "wq": wq, "bq": bqs,
            "srcidx": srcw,
            "Sh": Sh, "ShT": ShT,
        })
    return in_maps


def build(cfg: Cfg):
    N, IN, H, D, C = cfg.N, cfg.IN, cfg.H, cfg.D, cfg.C
    KA = cfg.KA
    C2 = 2 * C
    CZ = C + H
    sum_blk = sum(cfg.SBLK)
    sum_epb = sum_blk * 128
    bf = mybir.dt.bfloat16
    f32 = mybir.dt.float32
    fp8 = mybir.dt.float8e4

    nc = bacc.Bacc("TRN2", target_bir_lowering=False, debug=False,
                   num_swdge_queues=N_SWDGE_Q)
    hT = nc.dram_tensor("hT", [IN, cfg.NT], bf, kind="ExternalInput").ap()
    hTq = nc.dram_tensor("hTq", [IN, cfg.NPC], bf, kind="ExternalInput").ap()
    wkv = nc.dram_tensor("wkv", [IN, C2], bf, kind="ExternalInput").ap()
    bkv = nc.dram_tensor("bkv", [1, C2], bf, kind="ExternalInput").ap()
    wq = nc.dram_tensor("wq", [IN, C], bf, kind="ExternalInput").ap()
    bq = nc.dram_tensor("bq", [1, C], bf, kind="ExternalInput").ap()
    srcidx = nc.dram_tensor("srcidx", [128, sum_epb // 16], mybir.dt.int16,
                            kind="ExternalInput").ap()
    Sh_d = nc.dram_tensor("Sh", [128, sum_blk, 128], fp8,
                          kind="ExternalInput").ap()
    ShT_d = nc.dram_tensor("ShT", [128, sum_blk, 128], fp8,
                           kind="ExternalInput").ap()
    out = nc.dram_tensor("out", [cfg.NPC, C], f32, kind="ExternalOutput").ap()

    with tile.TileContext(nc) as tc:
        with (
            tc.tile_pool(name="dram", bufs=1, space="DRAM") as dramp,
            tc.tile_pool(name="const", bufs=1) as constp,
        ):
            kv_t = [dramp.tile([cfg.NTG, C2], bf, name=f"kv_t{g}")
                    for g in range(NGRP)]

            wkvt = constp.tile([128, KA, C2], bf)
            nc.sync.dma_start(wkvt[:], wkv.rearrange("(a p) c -> p a c", p=128))
            wqt = constp.tile([128, KA, C], bf)
            nc.sync.dma_start(wqt[:], wq.rearrange("(a p) c -> p a c", p=128))
            bkvt = constp.tile([1, C2], bf)
            nc.sync.dma_start(bkvt[:], bkv[:])
            bqt = constp.tile([1, C], bf)
            nc.sync.dma_start(bqt[:], bq[:])
            ones1 = constp.tile([1, 128], bf)
            nc.vector.memset(ones1[:], 1.0)
            srct = constp.tile([128, sum_epb // 16], mybir.dt.int16)
            nc.sync.dma_start(srct[:], srcidx[:])
            qs = constp.tile([128, cfg.NSB, C], bf)
            bias_kv = constp.tile([128, C2], bf)
            acc = constp.tile([128, cfg.NSB, CZ], f32)

            # ---------------- Phase A ----------------
            import contextlib
            pg_ctx = contextlib.ExitStack()
            pg = pg_ctx.enter_context(tc.tile_pool(name="pb_g", bufs=5))
            with (
                tc.tile_pool(name="pa_h", bufs=1) as pah,
                tc.tile_pool(name="pa_ps", bufs=6, space="PSUM") as paps,
                tc.tile_pool(name="pa_bps", bufs=1, space="PSUM") as pabps,
                tc.tile_pool(name="pa_sb", bufs=4) as pasb,
            ):
                bps = pabps.tile([128, C2], f32, tag="bps")
                nc.tensor.matmul(out=bps[:], lhsT=ones1[:], rhs=bkvt[:],
                                 start=True, stop=True)
                nc.vector.tensor_copy(bias_kv[:], bps[:])
                bpq = pabps.tile([128, C], f32, tag="bpq")
                nc.tensor.matmul(out=bpq[:], lhsT=ones1[:], rhs=bqt[:],
                                 start=True, stop=True)
                bias_q = pasb.tile([128, C], bf, tag="biasq")
                nc.vector.tensor_copy(bias_q[:], bpq[:])

                hts = pah.tile([128, KA, cfg.NT], bf)
                hT_r = hT.rearrange("(a p) n -> p a n", p=128)
                htq = pah.tile([128, KA, cfg.NPC], bf)
                nc.sync.dma_start(htq[:], hTq.rearrange("(a p) n -> p a n", p=128))
                NSPL = 8
                SPL = cfg.NT // NSPL
                for sp in range(NSPL):
                    nc.sync.dma_start(hts[:, :, ts(sp, SPL)], hT_r[:, :, ts(sp, SPL)])

                # Q projection first: unblocks the first Q-gather matmuls
                for qc in range(cfg.NSB):
                    psq = paps.tile([128, C], f32, tag="psA", name="psq")
                    for a in range(KA):
                        nc.tensor.matmul(out=psq[:], lhsT=htq[:, a, ts(qc, 128)],
                                         rhs=wqt[:, a, :], start=(a == 0),
                                         stop=(a == KA - 1))
                    nc.vector.tensor_tensor(qs[:, qc, :], psq[:], bias_q[:],
                                            op=ALU.add)

                NCG = cfg.NTG // 128

                def kv_chunk(cc):
                    ps = paps.tile([128, C2], f32, tag="psA")
                    for a in range(KA):
                        nc.tensor.matmul(out=ps[:], lhsT=hts[:, a, ts(cc, 128)],
                                         rhs=wkvt[:, a, :], start=(a == 0),
                                         stop=(a == KA - 1))
                    buf = pasb.tile([128, C2], bf, tag="bufA")
                    # K half raw (bk cancels in the per-dst softmax); V + bias
                    nc.scalar.copy(buf[:, 0:C], ps[:, 0:C])
                    nc.vector.tensor_tensor(buf[:, C:C2], ps[:, C:C2],
                                            bias_kv[:, C:C2], op=ALU.add)
                    nc.sync.dma_start(kv_t[cc // NCG][ts(cc % NCG, 128), :],
                                      buf[:])

                for cc in range(NGRP * NCG):
                    kv_chunk(cc)

            # ---------------- Phase B ----------------
            grp_off = {}
            off_b = 0
            for j in range(cfg.NSB):
                for g in range(NGRP):
                    grp_off[(j, g)] = off_b
                    off_b += cfg.CHG[j][g]

            with (
                tc.tile_pool(name="pb_t", bufs=2) as pt,
                tc.tile_pool(name="pb_c", bufs=2) as pc,
                tc.tile_pool(name="pb_w", bufs=2) as pw,
                tc.tile_pool(name="pb_s", bufs=2) as psm,
                tc.tile_pool(name="pb_ps", bufs=2, space="PSUM") as pps,
                tc.tile_pool(name="pb_qps", bufs=2, space="PSUM") as pqps,
            ):
                gq_counter = [0]
                plan_all = chunk_plan(cfg)

                def process_group(j, g):
                    gb = grp_off[(j, g)]
                    chunks = [(cb, CH, key) for (jj, gg, cb, CH, key) in plan_all
                              if jj == j and gg == g]
                    pswz = pps.tile([128, CZ], f32, tag="pswz")
                    for (ci, (cb, CH, key)) in enumerate(chunks):
                        cbk = gb + cb
                        ce = cbk * 128
                        EPC = CH * 128
                        kvg = pg.tile([128, CH, C2], bf, tag="kvg")
                        nc.gpsimd.dma_gather(
                            kvg[:], kv_t[g][:], srct[:, ce // 16:(ce + EPC) // 16],
                            EPC, EPC, C2, single_packet=False,
                            queue_num=1 + gq_counter[0] % 3)
                        gq_counter[0] += 1

                        # one-hot dst matrices (fp8, host-prepared)
                        sh = pt.tile([128, CH, 128], fp8, tag="sh")
                        nc.scalar.dma_start(sh[:], Sh_d[:, cbk:cbk + CH, :])
                        sht = pt.tile([128, CH, 128], fp8, tag="sht")
                        nc.scalar.dma_start(sht[:], ShT_d[:, cbk:cbk + CH, :])

                        # --- Q gather (PE) + PSUM->SBUF copy on ACT ---
                        qg = pc.tile([128, CH, C], bf, tag="qg")
                        for b0q in range(0, CH, QW):
                            bw = min(QW, CH - b0q)
                            qps = pqps.tile([128, QW, C], f32, tag="qps")
                            for b in range(b0q, b0q + bw):
                                nc.tensor.matmul(out=qps[:, b - b0q, :],
                                                 lhsT=sht[:, b, :],
                                                 rhs=qs[:, j, :],
                                                 start=True, stop=True)
                            nc.scalar.copy(qg[:, b0q:b0q + bw, :],
                                           qps[:, 0:bw, :])

                        # --- P = K * Qg, tree-reduce over D, exp ---
                        P = pc.tile([128, CH, C], bf, tag="P")
                        nc.vector.tensor_tensor(P[:], kvg[:, :, 0:C], qg[:],
                                                op=ALU.mult)
                        P4 = P[:].rearrange("p b (h d) -> p b h d", d=D)
                        cur = P4
                        w = D
                        while w > 2:
                            w //= 2
                            nxt = pc.tile([128, CH, H, w], bf, tag=f"s{w}")
                            nc.vector.tensor_tensor(
                                nxt[:], cur[:, :, :, 0:w], cur[:, :, :, w:2 * w],
                                op=ALU.add)
                            cur = nxt[:]
                        sc = pc.tile([128, CH, H], f32, tag="sc")
                        nc.vector.tensor_tensor(
                            sc[:].unsqueeze(3), cur[:, :, :, 0:1], cur[:, :, :, 1:2],
                            op=ALU.add)

                        wvz = pw.tile([128, CH, CZ], bf, tag="wvz")
                        nc.scalar.activation(wvz[:, :, C:CZ], sc[:], AF.Exp)
                        nc.vector.tensor_tensor(
                            wvz[:, :, 0:C].rearrange("p b (h d) -> p b h d", d=D),
                            kvg[:, :, C:C2].rearrange("p b (h d) -> p b h d", d=D),
                            wvz[:, :, C:CZ].unsqueeze(3)
                            .broadcast_to([128, CH, H, D]),
                            op=ALU.mult)

                        for b in range(CH):
                            nc.tensor.matmul(
                                out=pswz[:], lhsT=sh[:, b, :], rhs=wvz[:, b, :],
                                start=(ci == 0 and b == 0),
                                stop=(ci == len(chunks) - 1 and b == CH - 1))

                    # fold this group's partial sums into the SBUF accumulator
                    if g == 0:
                        nc.vector.tensor_copy(acc[:, j, :], pswz[:])
                    else:
                        nc.vector.tensor_tensor(acc[:, j, :], pswz[:],
                                                acc[:, j, :], op=ALU.add)

                def finalize(j):
                    zm = psm.tile([128, H], f32, tag="zm")
                    nc.vector.tensor_scalar(zm[:], acc[:, j, C:CZ], 1e-30, None,
                                            op0=ALU.max)
                    zr = psm.tile([128, H], f32, tag="zr")
                    nc.vector.reciprocal(zr[:], zm[:])
                    of = psm.tile([128, C], f32, tag="of")
                    nc.vector.tensor_tensor(
                        of[:].rearrange("p (h d) -> p h d", d=D),
                        acc[:, j, 0:C].rearrange("p (h d) -> p h d", d=D),
                        zr[:].unsqueeze(2).broadcast_to([128, H, D]),
                        op=ALU.mult)
                    nc.scalar.dma_start(out[ts(j, 128), :], of[:])

                for g in range(NGRP):
                    for j in range(cfg.NSB):
                        process_group(j, g)
                        if g == NGRP - 1:
                            finalize(j)
            pg_ctx.close()

    nc.compile()
    return nc


def run(cfg: Cfg, in_maps, trace=False, nc=None):
    if nc is None:
        nc = build(cfg)
    res = run_bass_kernel_spmd(nc, in_maps, core_ids=list(range(cfg.n_cores)),
                               trace=trace)
    full = np.zeros((cfg.NT, cfg.C), dtype=np.float32)
    for i in range(cfg.n_cores):
        o = res.results[i]["out"]
        for j in range(cfg.NSB):
            g_sb = cfg.ASSIGN[i][j]
            full[g_sb * 128:(g_sb + 1) * 128] = o[j * 128:(j + 1) * 128]
    full = full[:cfg.N]
    return full.reshape(cfg.N, cfg.H, cfg.D), res


_PROBLEM_N = 10000
_PROBLEM_IN = 256
_PROBLEM_H = 8
_PROBLEM_D = 32


def kernel(h, Wq, bq, Wk, bk, Wv, bv, src, dst):
    h = np.asarray(h)
    N, IN = h.shape
    C = np.asarray(Wq).shape[1]
    H, D = _PROBLEM_H, _PROBLEM_D
    if C != H * D:
        D = C // H
    src = np.asarray(src)
    dst = np.asarray(dst)
    cfg = make_cfg(N, IN, H, D, src, dst)
    in_maps = prep(cfg, h, Wq, bq, Wk, bk, Wv, bv, src, dst)
    out, _ = run(cfg, in_maps, trace=False)
    return out.astype(np.float32)
